# revision 49
# baseline (speedup 1.0000x reference)
"""BrushStroke splat kernel for 8 trn2 NeuronCores.

out[b,c,y,x] = mean_n sum_{p,q} Fy[b,n,y,p] Fx[b,n,x,q] patches[b,n,c,p,q]
with Fx/Fy separable Gaussian filter banks (sigma=0.1) normalized over a
padded spatial axis.

Strategy (per core, 2 batches of 64 strokes), v3 — no DMA gathers, no
gpsimd in the steady state:
 - Per group of 4 strokes, one Derivative_Erf activation per axis
   evaluates (2/sqrt(pi)) * exp(-((t + q') - (g + 31.5))^2 / (2 s^2))
   on a [128(j,q'), 288] iota tile using a per-partition bias built once
   via a one-hot matmul (the 2/sqrt(pi) factor cancels in normalizers).
 - All filter normalizers are precomputed once: E rows per stroke
   [128, 319] -> cumsum scan -> window sums W -> reciprocal -> remapped
   to the per-group [(j,q'), (b,g)] layout with a masked one-hot matmul.
 - MM1 per group: 3 bf16 matmuls (block-diagonal patch lhsT) into one
   PSUM span [128, 768]; a single DVE drain rescales by 1/Wy into bf16.
 - MM2 per group: 4 bf16 matmuls (2 y-tiles x {c0c1 merged, c2}) chained
   over the 16 groups into 4 single-bank PSUM accumulators; unnormalized
   Fy rows are the stationary. MM2 for group g is emitted one iteration
   late so the tensor engine never waits on the drain.
Batch-parallel across cores; no collectives.
"""
import sys, types
import numpy as np

IMAGE = 256
PAD = 16
EPS = 1e-7
SIGMA2 = 2.0 * 0.1 ** 2
B, N, C, PH, PW = 16, 64, 3, 32, 32
NCORES = 8
BLOC = B // NCORES          # 2 batches per core
NG = N // 4                 # 16 groups of 4 strokes
W288 = IMAGE + 2 * PAD      # padded spatial axis length
SCL = (1.0 / SIGMA2) ** 0.5  # derf(SCL*t + SCL*b) ~ exp(-(t+b)^2/SIGMA2)
CX = PW / 2 - 0.5 + PAD      # 31.5
CY = PW / 2 - 0.4 + PAD      # 31.6


def _install_patches():
    if 'antenv.axon_hooks' not in sys.modules:
        mod = types.ModuleType('antenv.axon_hooks')
        mod._hook = None
        mod.set_axon_ntff_profile_hook = lambda h: setattr(mod, '_hook', h)
        mod.get_axon_ntff_profile_hook = lambda: mod._hook
        sys.modules['antenv.axon_hooks'] = mod
        try:
            from trn_agent_boot.trn_boot import _ntff_profile_via_ctypes
            hook = _ntff_profile_via_ctypes('/opt/axon/libaxon_pjrt.so')
            if hook is not None:
                mod.set_axon_ntff_profile_hook(hook)
        except Exception:
            pass

    import concourse.tile as tile
    import concourse.bass_utils as bass_utils
    from concourse.vector_clock import ScopedClock

    bass_utils.upload_artifacts = lambda tmpdir: 'local://' + tmpdir

    if getattr(tile.TileContext._drain_and_barrier, '_patched', False):
        return

    def _drain_and_barrier(self, tick_clock, wait_clock):
        nc = self.nc
        drain_inst = nc.sync.drain()
        wait_clock.add_sem_waits(
            drain_inst.ins, ScopedClock({None: tick_clock.global_clock}))
        si = drain_inst.ins.sync_info
        waits = list(si.on_wait or [])
        si.on_wait = []
        for w in waits:
            nop = nc.sync.nop()
            nop.ins.sync_info = type(si)(on_wait=[w], on_update=[])
        nc.all_engine_barrier()
        popped = nc._tile_sem_poison_stack.pop()
        assert popped is self._sem_poison
        nc.clear_and_free_semaphores(list(self.sems.allocated().values()))
        nc.all_engine_barrier()

    _drain_and_barrier._patched = True
    tile.TileContext._drain_and_barrier = _drain_and_barrier


def _split_multi_waits(nc):
    """This walrus accepts at most one sync wait per instruction; hoist
    extras onto same-engine NoOps inserted just before."""
    import bass_rust
    n_new = [0]

    def fresh_nop(engine, wait, si_type):
        n_new[0] += 1
        nop = bass_rust.InstNoOp(name=f'I-waitsplit-{n_new[0]}', ins=[], outs=[])
        nop.engine = engine
        nop.sync_info = si_type(on_wait=[wait], on_update=[])
        return nop

    for fn in nc.m.functions:
        for blk in fn.blocks:
            insts = blk.instructions
            i = 0
            while i < len(insts):
                inst = insts[i]
                si = inst.sync_info
                if si is not None and si.on_wait and len(si.on_wait) > 1:
                    waits = list(si.on_wait)
                    si.on_wait = [waits[-1]]
                    for k, w in enumerate(waits[:-1]):
                        insts.insert(i + k, fresh_nop(inst.engine, w, type(si)))
                    i += len(waits) - 1
                i += 1


_PROGRAM = None


def _build_program():
    global _PROGRAM
    if _PROGRAM is not None:
        return _PROGRAM
    _install_patches()
    import concourse.bass as bass
    import concourse.tile as tile
    from concourse import mybir
    from bass_rust import AP

    f32 = mybir.dt.float32
    bf16 = mybir.dt.bfloat16
    AF = mybir.ActivationFunctionType
    AX = mybir.AxisListType
    ALU = mybir.AluOpType
    MUL, SUB = ALU.mult, ALU.subtract

    nc = bass.Bass('TRN2', target_bir_lowering=False, debug=False,
                   num_devices=NCORES)
    # brush coords padded so both batches sit at PE-legal partition bases:
    # rows 0,1 = b0 x,y; rows 32,33 = b1 x,y (others a harmless constant)
    g_in = nc.declare_dram_parameter('g_in', [34, N], f32, isOutput=False)
    pt_in = nc.declare_dram_parameter('pt_in', [BLOC, 128, NG * C * PH], bf16,
                                      isOutput=False)
    id34 = nc.declare_dram_parameter('id34', [34, 2], f32, isOutput=False)
    mask4 = nc.declare_dram_parameter('mask4', [N, 128], f32, isOutput=False)
    mask16 = nc.declare_dram_parameter('mask16', [N, NG], f32, isOutput=False)
    mask4f2 = nc.declare_dram_parameter('mask4f2', [128, 256], f32,
                                        isOutput=False)
    mask32 = nc.declare_dram_parameter('mask32', [128, 32], f32,
                                       isOutput=False)
    selab = nc.declare_dram_parameter('selab', [N, 256], f32,
                                      isOutput=False)
    qp2_in = nc.declare_dram_parameter('qp2_in', [128, 64], f32,
                                       isOutput=False)
    y_out = nc.declare_dram_parameter('y_out', [BLOC, C, IMAGE, IMAGE], f32,
                                      isOutput=True)

    with tile.TileContext(nc) as tc:
        with tc.tile_pool(name='glob', bufs=1) as gp, \
             tc.tile_pool(name='work', bufs=1) as wp, \
             tc.tile_pool(name='ps1', bufs=2, space='PSUM') as ps1, \
             tc.tile_pool(name='ps2', bufs=1, space='PSUM') as ps2:
            # accumulators: one PSUM bank per chain (interleaved
            # accumulation groups sharing a bank corrupt each other).
            # Preamble PSUM results squat in their unused columns.
            a01t = [ps2.tile([128, 512], f32, name=f'a01_{yt}')
                    for yt in range(2)]
            a2t = [ps2.tile([128, 256], f32, name=f'a2_{yt}')
                   for yt in range(2)]

            # ---- iota ramp 0..319 (gpsimd, preamble only) ----
            it = gp.tile([128, 320], f32)
            nc.gpsimd.iota(it[:], pattern=[[1, 320]], base=0,
                           channel_multiplier=0,
                           allow_small_or_imprecise_dtypes=True)

            # ---- input DMAs, spread across the 3 DMA-capable queues ----
            bc = gp.tile([34, N], f32)
            nc.sync.dma_start(bc[:], g_in[:])
            idt = gp.tile([34, 2], f32)
            nc.sync.dma_start(idt[:], id34[:])
            m16 = gp.tile([N, NG], f32)
            nc.scalar.dma_start(m16[:], mask16[:])
            m4 = gp.tile([N, 128], f32)
            nc.scalar.dma_start(m4[:], mask4[:])
            qp2 = gp.tile([128, 64], f32)
            nc.scalar.dma_start(qp2[:], qp2_in[:])
            m4f2 = gp.tile([128, 256], f32)
            nc.scalar.dma_start(m4f2[:], mask4f2[:])
            m32 = gp.tile([128, 32], f32)
            nc.scalar.dma_start(m32[:], mask32[:])
            selAB = gp.tile([N, 256], f32)
            nc.sync.dma_start(selAB[:], selab[:])
            ptc = []
            for b in range(BLOC):
                t_ = gp.tile([128, NG * C * PH], bf16, name=f'ptc{b}')
                [nc.gpsimd, nc.sync][b].dma_start(t_[:], pt_in[b])
                ptc.append(t_)

            # ---- block-diagonal patch lhsT (zeros + copies) ----
            ps_all = [gp.tile([128, 128 * C * NG], bf16, name=f'psall{b}')
                      for b in range(BLOC)]

            def emit_psall_memset(b, lo, hi, eng):
                v = ps_all[b].bitcast(f32)
                eng.memset(v[:, lo:hi], 0.0)

            def emit_psall_copy(b, j, eng):
                dst0 = ps_all[b][32 * j:32 * j + 1, 32 * j:32 * j + 1]
                dst = AP(ps_all[b].tensor, dst0.offset,
                         [[128 * C * NG, 32], [128 * C, NG],
                          [128, C], [1, PH]])
                src0 = ptc[b][32 * j:32 * j + 1, 0:1]
                srcap = AP(ptc[b].tensor, src0.offset,
                           [[NG * C * PH, 32], [C * PH, NG],
                            [PH, C], [1, PH]])
                if eng is nc.scalar:
                    eng.copy(dst, srcap)
                else:
                    eng.tensor_copy(dst, srcap)

            emit_psall_memset(0, 0, 1536, nc.gpsimd)
            emit_psall_memset(0, 1536, 3072, nc.gpsimd)

            # ---- brush normalization on the padded [34, N] layout ----
            mn = gp.tile([34, 1], f32)
            mx = gp.tile([34, 1], f32)
            nc.vector.tensor_reduce(mn[:], bc[:], axis=AX.X, op=ALU.min)
            nc.vector.reduce_max(mx[:], bc[:], axis=AX.X)
            rng = gp.tile([34, 1], f32)
            nc.vector.tensor_sub(rng[:], mx[:], mn[:])
            nc.vector.tensor_scalar_add(rng[:], rng[:], EPS)
            inv = gp.tile([34, 1], f32)
            nc.vector.reciprocal(inv[:], rng[:])
            nc.vector.tensor_scalar_mul(inv[:], inv[:], float(IMAGE))
            gn = gp.tile([34, N], f32)
            nc.vector.tensor_scalar_sub(gn[:], bc[:], mn[:])
            nc.vector.tensor_scalar_mul(gn[:], gn[:], inv[:])

            # two tiny PE transposes give tp (strokes x b0x,b0y,b1x,b1y)
            # squatting in a2t[0][:, 64:68]
            pre = a2t[0]
            tp_ps = pre[0:N, 64:68]
            nc.tensor.transpose(pre[0:N, 64:66], gn[0:2, :], idt[0:2, 0:2])
            nc.tensor.transpose(pre[0:N, 66:68], gn[32:34, :],
                                idt[32:34, 0:2])
            # negT = -SCL * tp feeds both rhsA and (via a select-matmul
            # stacking both batches onto 128 partitions) the E-row biases
            negT = gp.tile([N, 4], f32)
            nc.vector.tensor_scalar_mul(negT[:], tp_ps, -SCL)
            psBE = a2t[1][:, 0:2]
            nc.tensor.matmul(psBE, selAB[:, 0:128], negT[:, 0:2],
                             start=True, stop=False)
            nc.tensor.matmul(psBE, selAB[:, 128:256], negT[:, 2:4],
                             start=False, stop=True)
            biasE = gp.tile([128, 2], f32)
            nc.vector.tensor_scalar_add(biasE[:, 0:1], psBE[:, 0:1],
                                        -SCL * CX)
            nc.vector.tensor_scalar_add(biasE[:, 1:2], psBE[:, 1:2],
                                        -SCL * CY)
            # rhsA[n, 16k+g] = mask16[n,g] * (-SCL * tp[n,k]) in one op
            rhsA = gp.tile([N, 64], f32)
            m16d = AP(m16.tensor, m16[0:1, 0:1].offset,
                      [[NG, N], [0, 4], [1, NG]])
            negTd = AP(negT.tensor, negT[0:1, 0:1].offset,
                       [[4, N], [1, 4], [0, NG]])
            nc.vector.tensor_tensor(rhsA[:], m16d, negTd, op=MUL)
            # BiasAll[32j+q', 16k+g] = SCL*(q' - g_{4g+j,k} - C(k));
            # the -SCL*C(k) and SCL*q' terms come in via the qp2 constant
            psB = pre[:, 0:64]
            nc.tensor.matmul(psB, m4[:], rhsA[:], start=True, stop=True)
            BiasAll = gp.tile([128, 64], f32)
            nc.vector.tensor_tensor(BiasAll[:], psB, qp2[:], op=ALU.add)

            # ---- E rows -> window sums W (reduce + short recurrence) ----
            E = gp.tile([128, 638], f32)
            emit_psall_copy(0, 0, nc.scalar)
            nc.scalar.activation(E[:, 0:319], it[:, 0:319],
                                 AF.Derivative_Erf,
                                 bias=biasE[:, 0:1], scale=SCL)
            emit_psall_copy(0, 1, nc.scalar)
            nc.scalar.activation(E[:, 319:638], it[:, 0:319],
                                 AF.Derivative_Erf,
                                 bias=biasE[:, 1:2], scale=SCL)
            emit_psall_copy(0, 2, nc.scalar)
            emit_psall_copy(0, 3, nc.scalar)
            # W[q'+1] = W[q'] + E[q'+288] - E[q'] (31 steps per axis)
            Wxy = gp.tile([128, 64], f32)
            nc.vector.reduce_sum(Wxy[:, 0:1], E[:, 0:288], axis=AX.X)
            nc.vector.tensor_tensor_scan(Wxy[:, 1:32], E[:, 288:319],
                                         E[:, 0:31], Wxy[:, 0:1],
                                         ALU.add, SUB)
            nc.vector.reduce_sum(Wxy[:, 32:33], E[:, 319:607], axis=AX.X)
            nc.vector.tensor_tensor_scan(Wxy[:, 33:64], E[:, 607:638],
                                         E[:, 319:350], Wxy[:, 32:33],
                                         ALU.add, SUB)
            # reference adds EPS to the normalizer (E rows carry 2/sqrt(pi));
            # the 1/N fold for the x block lives in the mask4f2 constant
            nc.vector.tensor_scalar_add(Wxy[:], Wxy[:],
                                        (2.0 / np.pi ** 0.5) * EPS)
            Winv = gp.tile([128, 64], f32)
            nc.vector.reciprocal(Winv[:], Wxy[:])
            # masked remap: IvAll[32j+q', (ax,16b+g)] = Winv[64b+4g+j,
            # (ax,q')] (x scaled by 1/N via mask4f2); the tiled replication
            # is materialized by a small SBUF->SBUF DMA so the mask
            # multiply stays on the DVE fast path
            winvt = gp.tile([128, 256], f32)
            for ax in range(2):
                wt = AP(Winv.tensor, Winv[0:1, 32 * ax:32 * ax + 1].offset,
                        [[64, 128], [0, 4], [1, 32]])
                nc.gpsimd.dma_start(winvt[:, 128 * ax:128 * (ax + 1)], wt)
            Wm = gp.tile([128, 256], f32)
            nc.vector.tensor_tensor(Wm[:], m4f2[:], winvt[:], op=MUL)
            nc.tensor.matmul(a01t[0][:, 0:32], Wm[:, 0:128], m32[:],
                             start=True, stop=True)
            nc.tensor.matmul(a01t[0][:, 32:64], Wm[:, 128:256], m32[:],
                             start=True, stop=True)
            ivA = gp.tile([128, 64], f32)
            nc.vector.tensor_copy(ivA[:], a01t[0][:, 0:64])

            # ---- main loop: one-group software pipeline on every engine
            # (drain and MM2 for group k-1 are emitted during iteration k,
            # after fxn/MM1 of group k, so no in-order stream ever blocks
            # on a cross-engine producer) ----
            prev = None
            NTOT = BLOC * NG
            for k in range(NTOT + 1):
                if k < NTOT:
                    b, g = divmod(k, NG)
                    colx, coly = 32 * b + g, 32 * b + 16 + g
                    civ = 16 * b + g
                    fx = wp.tile([128, W288], bf16, name='fx', tag='fx',
                                 bufs=3)
                    nc.scalar.activation(fx[:], it[:, 0:W288],
                                         AF.Derivative_Erf,
                                         bias=BiasAll[:, colx:colx + 1],
                                         scale=SCL)
                    fy = wp.tile([128, W288], bf16, name='fy', tag='fy',
                                 bufs=4)
                    nc.scalar.activation(fy[:], it[:, 0:W288],
                                         AF.Derivative_Erf,
                                         bias=BiasAll[:, coly:coly + 1],
                                         scale=SCL)
                    fxn = wp.tile([128, IMAGE], bf16, name='fxn', tag='fxn',
                                  bufs=3)
                    nc.vector.tensor_scalar_mul(fxn[:],
                                                fx[:, PAD:PAD + IMAGE],
                                                ivA[:, civ:civ + 1])
                    # MM1: 3 channels into one PSUM span
                    pfull = ps1.tile([128, 768], f32, name='pfull',
                                     tag='pfull')
                    for c in range(C):
                        nc.tensor.matmul(
                            pfull[:, 256 * c:256 * (c + 1)],
                            ps_all[b][:, 384 * g + 128 * c:
                                      384 * g + 128 * (c + 1)],
                            fxn[:], start=True, stop=True)
                    cur = (b, g, fy, pfull, civ)
                else:
                    cur = None

                # batch-1 lhsT build on gpsimd during early iterations
                if k == 0:
                    emit_psall_memset(1, 0, 1536, nc.gpsimd)
                elif k == 1:
                    emit_psall_memset(1, 1536, 3072, nc.gpsimd)
                elif k in (2, 3, 4, 5):
                    emit_psall_copy(1, k - 2, nc.gpsimd)

                if prev is not None:
                    pb, pg, pfy, ppfull, pciv = prev
                    # drain of group k-1 (DVE), rescaled by 1/Wy
                    tall = wp.tile([128, 768], bf16, name='tall', tag='tall',
                                   bufs=3)
                    nc.vector.tensor_scalar_mul(tall[:], ppfull[:],
                                                ivA[:, 32 + pciv:33 + pciv])
                    for yt in range(2):
                        fyv = pfy[:, PAD + 128 * yt:PAD + 128 * yt + 128]
                        nc.tensor.matmul(a01t[yt][:], fyv, tall[:, 0:512],
                                         start=(pg == 0), stop=(pg == NG - 1))
                        nc.tensor.matmul(a2t[yt][:], fyv, tall[:, 512:768],
                                         start=(pg == 0), stop=(pg == NG - 1))
                    if pg == NG - 1:
                        ob01 = [wp.tile([128, 512], f32, name=f'ob01_{yt}',
                                        tag=f'ob01_{yt}', bufs=1)
                                for yt in range(2)]
                        ob2 = wp.tile([128, 512], f32, name='ob2', tag='ob2',
                                      bufs=1)
                        nc.scalar.copy(ob01[0][:], a01t[0][:])
                        nc.vector.tensor_copy(ob01[1][:], a01t[1][:])
                        nc.scalar.copy(ob2[:, 0:256], a2t[0][:])
                        nc.vector.tensor_copy(ob2[:, 256:512], a2t[1][:])
                        qs = [nc.sync, nc.scalar, nc.sync,
                              nc.scalar, nc.sync, nc.scalar]
                        for yt in range(2):
                            qs[3 * yt].dma_start(
                                y_out[pb, 0, 128 * yt:128 * (yt + 1), :],
                                ob01[yt][:, 0:256])
                            qs[3 * yt + 1].dma_start(
                                y_out[pb, 1, 128 * yt:128 * (yt + 1), :],
                                ob01[yt][:, 256:512])
                            qs[3 * yt + 2].dma_start(
                                y_out[pb, 2, 128 * yt:128 * (yt + 1), :],
                                ob2[:, 256 * yt:256 * (yt + 1)])
                prev = cur

    _split_multi_waits(nc)
    _PROGRAM = nc
    return nc


def _make_in_maps(brushes: np.ndarray, patches: np.ndarray):
    import ml_dtypes
    brushes = np.asarray(brushes, dtype=np.float32)
    patches = np.asarray(patches, dtype=np.float32)
    id34 = np.zeros((34, 2), dtype=np.float32)
    id34[0:2] = np.eye(2)
    id34[32:34] = np.eye(2)
    nn = np.arange(N)
    # mask4[n, 32j+q'] = 1 where j == n % 4, for every q'
    mask4 = np.zeros((N, 128), dtype=np.float32)
    for n in range(N):
        j = n % 4
        mask4[n, 32 * j:32 * (j + 1)] = 1.0
    mask16 = np.zeros((N, NG), dtype=np.float32)
    mask16[nn, nn // 4] = 1.0
    mask4f = np.zeros((128, 128), dtype=np.float32)
    for n in range(128):
        j = n % 4
        mask4f[n, 32 * j:32 * (j + 1)] = 1.0
    # x half folds the 1/N mean scale; y half is the plain mask
    mask4f2 = np.concatenate([mask4f / N, mask4f], axis=1)
    mask32 = np.zeros((128, 32), dtype=np.float32)
    mask32[np.arange(128), np.arange(128) // 4] = 1.0
    # selAB[k, p] / [k, 128+p]: stack b0 (p<64) and b1 (p>=64) strokes
    selab = np.zeros((N, 256), dtype=np.float32)
    selab[np.arange(N), np.arange(N)] = 1.0
    selab[np.arange(N), 128 + 64 + np.arange(N)] = 1.0
    # qp2[p, 16k+g] = SCL*(q'(p) - C(k)), C = CX for x cols, CY for y
    qprime = (np.arange(128) % 32).astype(np.float32)
    cks = np.array([CX, CY, CX, CY], dtype=np.float32)
    qp2 = SCL * (qprime[:, None] - np.repeat(cks, NG)[None, :])
    in_maps = []
    for k in range(NCORES):
        bsl = brushes[BLOC * k: BLOC * (k + 1)]        # [2, 64, 2]
        g4 = bsl.transpose(0, 2, 1).reshape(4, N)       # rows b0x,b0y,b1x,b1y
        g_in = np.full((34, N), 0.5, dtype=np.float32)
        g_in[0:2] = g4[0:2]
        g_in[32:34] = g4[2:4]
        psl = patches[BLOC * k: BLOC * (k + 1)]         # [2, 64, 3, 32, 32]
        pr = psl.reshape(BLOC, NG, 4, C, PH, PW)[..., ::-1, ::-1]
        # -> [b, j, q', g, c, p'] -> [b, 128, NG*C*PH]
        pt = np.ascontiguousarray(pr.transpose(0, 2, 5, 1, 3, 4)).reshape(
            BLOC, 128, NG * C * PH).astype(ml_dtypes.bfloat16)
        in_maps.append({'g_in': g_in, 'pt_in': pt, 'id34': id34,
                        'mask4': mask4, 'mask16': mask16,
                        'mask4f2': mask4f2.astype(np.float32),
                        'mask32': mask32, 'selab': selab,
                        'qp2_in': qp2.astype(np.float32)})
    return in_maps


def kernel(brushes: np.ndarray, patches: np.ndarray) -> np.ndarray:
    from concourse.bass_utils import run_bass_kernel_spmd

    nc = _build_program()
    in_maps = _make_in_maps(brushes, patches)
    res = run_bass_kernel_spmd(nc, in_maps, list(range(NCORES)))
    out = np.concatenate([res.results[k]['y_out'] for k in range(NCORES)],
                         axis=0)
    return out
